# revision 13
# baseline (speedup 1.0000x reference)
"""Bass/Trainium2 kernel for nn_BaseODERNN (ODE-RNN: RK4 ODE solve + GRUCell + fc per step).

Strategy:
  - Pure data parallel over batch B=2048 -> 8 cores x 256.
  - Per core, batch is optionally split into NS interleaved "streams" whose
    dependency chains fill each other's engine-latency gaps.
  - Everything is kept in [feature, batch] layout so H=128 sits on SBUF
    partitions; x is pre-transposed on the host, output is produced transposed
    and fixed up on the host.
  - RK4 stage algebra is folded:
      u_1 = w1 @ h + b1
      u_{i+1} = u_1 + c_i * (W12 @ a_i + w1 @ b2),   W12 = w1 @ w2, a_i = tanh(u_i)
      h   += sum_i d_i * (w2 @ a_i + b2)
    so each stage is one PSUM-accumulated matmul + one tanh (bias folded into
    the ACT bias vector / augmented ones-row of a_i).
  - GRU: gi (from x_t) and gh (from h) accumulate into shared PSUM banks per
    gate; sigmoid/tanh read PSUM directly with folded biases.
  - NOTE: matmul start=True clears the WHOLE psum bank -> exactly one
    start=True per bank "era".
  - Matmuls optionally run as float32r (bitcast views): at moving-dim 256 the
    PE streams 1 cycle/col vs 4 for fp32.
"""

import os

import numpy as np

import concourse.bass as bass
import concourse.bacc as bacc
import concourse.mybir as mybir
from concourse import tile
from concourse.bass_utils import run_bass_kernel_spmd

F32 = mybir.dt.float32
F32R = mybir.dt.float32r
AF = mybir.ActivationFunctionType
ALU = mybir.AluOpType

T_FULL, B_FULL, D_IN, H, NC_OUT = 200, 2048, 64, 128, 32
MLP_H = 50
N_SUB = 4
N_CORES = 8
B_LOC = B_FULL // N_CORES   # 256
TS_FULL = T_FULL - 1        # 199 scan steps

NS = int(os.environ.get("K_NS", "1"))       # streams per core
USE_F32R = os.environ.get("K_F32R", "1") == "1"
BW = B_LOC // NS

LAST_EXEC_NS = None

_BUILT = {}


def _build_nc(ts, use_bhhn):
    nc = bacc.Bacc(
        "TRN2",
        target_bir_lowering=False,
        debug=False,
        num_devices=N_CORES,
        enable_asserts=False,
    )

    d = {}

    MMDT_D = F32R if USE_F32R else F32

    def din(name, shape, dt_=F32):
        d[name] = nc.dram_tensor(name, list(shape), dt_, kind="ExternalInput").ap()

    din("xT", (ts, D_IN, B_LOC), MMDT_D)
    din("w1T", (H, MLP_H), MMDT_D)
    din("w12c2", (MLP_H + 1, MLP_H), MMDT_D)
    din("w12c4", (MLP_H + 1, MLP_H), MMDT_D)
    din("w2d1", (MLP_H + 1, H), MMDT_D)
    din("w2d2", (MLP_H + 1, H), MMDT_D)
    din("whhT", (H, 3 * H), MMDT_D)
    din("wihT", (D_IN, 3 * H), MMDT_D)
    din("fcT", (H, NC_OUT), MMDT_D)
    din("b1v", (MLP_H, 1))
    din("rbias", (H, 1))
    din("zbias", (H, 1))
    din("nbias", (H, 1))
    din("bhhn", (H, 1))
    din("fcb", (NC_OUT, 1))
    din("ones32", (32, BW), MMDT_D)
    din("zerosH", (H, BW), MMDT_D)
    outT = nc.dram_tensor("outT", [ts, NC_OUT, B_LOC], F32, kind="ExternalOutput").ap()

    MMDT = F32R if USE_F32R else F32

    def mm(out, lhsT, rhs, start, stop):
        nc.tensor.matmul(out, lhsT, rhs, start=start, stop=stop)

    with tile.TileContext(nc) as tc:
        with (
            tc.tile_pool(name="const", bufs=1) as cpool,
            tc.tile_pool(name="xtp", bufs=2) as xpool,
            tc.tile_pool(name="hp", bufs=2) as hpool,
            tc.tile_pool(name="work", bufs=2) as wpool,
            tc.tile_pool(name="outp", bufs=3) as opool,
            tc.tile_pool(name="ps", bufs=1, space=bass.MemorySpace.PSUM) as pspool,
        ):
            def const_tile(name, shape, dt_=F32):
                t_ = cpool.tile(list(shape), dt_, tag=name, name=name)
                nc.sync.dma_start(out=t_[:], in_=d[name][:])
                return t_

            w1T = const_tile("w1T", (H, MLP_H), MMDT)
            w12c2 = const_tile("w12c2", (MLP_H + 1, MLP_H), MMDT)
            w12c4 = const_tile("w12c4", (MLP_H + 1, MLP_H), MMDT)
            w2d1 = const_tile("w2d1", (MLP_H + 1, H), MMDT)
            w2d2 = const_tile("w2d2", (MLP_H + 1, H), MMDT)
            whhT = const_tile("whhT", (H, 3 * H), MMDT)
            wihT = const_tile("wihT", (D_IN, 3 * H), MMDT)
            fcT = const_tile("fcT", (H, NC_OUT), MMDT)
            b1v = const_tile("b1v", (MLP_H, 1))
            rbias = const_tile("rbias", (H, 1))
            zbias = const_tile("zbias", (H, 1))
            nbias = const_tile("nbias", (H, 1))
            bhhn = const_tile("bhhn", (H, 1))
            fcb = const_tile("fcb", (NC_OUT, 1))

            # per-stream persistent a-tiles with a constant ones-row (bias row)
            atiles = []
            for s in range(NS):
                row = []
                for i in range(4):
                    a_ = cpool.tile([64, BW], MMDT, tag=f"a{i}s{s}", name=f"a{i}s{s}")
                    # ones "bias row" at partition 50 via DMA (memset can't target
                    # f32r and needs 32-aligned bases): rows [32:64) get 1.0;
                    # tanh rewrites [0:50) and rows 51+ are never read.
                    nc.sync.dma_start(out=a_[32:64, :], in_=d["ones32"][:])
                    row.append(a_)
                atiles.append(row)

            V = [
                [pspool.tile([MLP_H, BW], F32, tag=f"V{j}s{s}", name=f"V{j}s{s}")
                 for j in range(2)]
                for s in range(NS)
            ]
            psafc = [pspool.tile([H, 2 * BW], F32, tag=f"pa{s}", name=f"pa{s}")
                     for s in range(NS)]
            # GRU gate psum regions: 4 x [H, BW] per stream.
            # BW=128: all four fit in one bank (one start=True era per step).
            # BW=256: two banks (r|z and ghn|gin), each with its own era.
            # flags per region: (gi_start, gi_stop, gh_start, gh_stop)
            gregs = []
            for s in range(NS):
                if BW == 128:
                    # all four regions share one bank; gi_n's start=True is the
                    # single whole-bank-clearing era start
                    g = pspool.tile([H, 4 * BW], F32, tag=f"g{s}", name=f"g{s}")
                    gregs.append({
                        "r": (g[:, 0:BW], False, False, False, False),
                        "z": (g[:, BW:2*BW], False, False, False, True),
                        "ghn": (g[:, 2*BW:3*BW], False, False, False, False),
                        "gin": (g[:, 3*BW:4*BW], True, False, None, None),
                    })
                else:
                    # one bank per gate region (sim- and hw-clean)
                    grR = pspool.tile([H, BW], F32, tag=f"grR{s}", name=f"grR{s}")
                    grZ = pspool.tile([H, BW], F32, tag=f"grZ{s}", name=f"grZ{s}")
                    grN = pspool.tile([H, BW], F32, tag=f"grN{s}", name=f"grN{s}")
                    grI = pspool.tile([H, BW], F32, tag=f"grI{s}", name=f"grI{s}")
                    gregs.append({
                        "r": (grR[:], True, False, False, True),
                        "z": (grZ[:], True, False, False, True),
                        "ghn": (grN[:], True, True, None, None),
                        "gin": (grI[:], True, True, None, None),
                    })

            # hidden state, zero-initialized
            h = []
            for s in range(NS):
                h0 = hpool.tile([H, BW], MMDT, tag=f"h{s}", name=f"h{s}")
                nc.sync.dma_start(out=h0[:], in_=d["zerosH"][:])
                h.append(h0)

            xt_cur = xpool.tile([D_IN, B_LOC], MMDT, tag="xt", name="xt")
            nc.sync.dma_start(out=xt_cur[:], in_=d["xT"][0])

            def stream_step(s, t, xt):
                o = s * BW
                a = atiles[s]
                va, vb = V[s]
                pa = psafc[s]
                gr = gregs[s]
                rR, rZ, rGHN, rGIN = gr["r"][0], gr["z"][0], gr["ghn"][0], gr["gin"][0]

                # gi matmuls: the designated region starts its bank's era
                mm(rGIN, wihT[:, 2 * H : 3 * H], xt[:, o : o + BW],
                   gr["gin"][1], gr["gin"][2])
                mm(rR, wihT[:, 0:H], xt[:, o : o + BW], gr["r"][1], gr["r"][2])
                mm(rZ, wihT[:, H : 2 * H], xt[:, o : o + BW], gr["z"][1], gr["z"][2])
                yield

                for _k in range(N_SUB):
                    mm(va[:], w1T[:], h[s][:], True, True)
                    nc.scalar.activation(a[0][0:MLP_H, :], va[:], AF.Tanh, bias=b1v[:])
                    yield
                    mm(vb[:], w1T[:], h[s][:], True, False)
                    mm(vb[:], w12c2[:], a[0][0 : MLP_H + 1, :], False, True)
                    mm(pa[:, 0:BW], w2d1[:], a[0][0 : MLP_H + 1, :], True, False)
                    mm(va[:], w1T[:], h[s][:], True, False)      # stage3 base
                    nc.scalar.activation(a[1][0:MLP_H, :], vb[:], AF.Tanh, bias=b1v[:])
                    yield
                    mm(va[:], w12c2[:], a[1][0 : MLP_H + 1, :], False, True)
                    mm(pa[:, 0:BW], w2d2[:], a[1][0 : MLP_H + 1, :], False, False)
                    mm(vb[:], w1T[:], h[s][:], True, False)      # stage4 base
                    nc.scalar.activation(a[2][0:MLP_H, :], va[:], AF.Tanh, bias=b1v[:])
                    yield
                    mm(vb[:], w12c4[:], a[2][0 : MLP_H + 1, :], False, True)
                    mm(pa[:, 0:BW], w2d2[:], a[2][0 : MLP_H + 1, :], False, False)
                    nc.scalar.activation(a[3][0:MLP_H, :], vb[:], AF.Tanh, bias=b1v[:])
                    yield
                    mm(pa[:, 0:BW], w2d1[:], a[3][0 : MLP_H + 1, :], False, True)
                    hn = hpool.tile([H, BW], MMDT, tag=f"h{s}", name=f"h{s}")
                    nc.vector.tensor_add(hn[:], h[s][:], pa[:, 0:BW])
                    h[s] = hn
                    yield

                # GRU
                mm(rGHN, whhT[:, 2 * H : 3 * H], h[s][:],
                   gr["ghn"][1], gr["ghn"][2])                           # gh_n
                mm(rR, whhT[:, 0:H], h[s][:], gr["r"][3], gr["r"][4])    # gh_r
                mm(rZ, whhT[:, H : 2 * H], h[s][:], gr["z"][3], gr["z"][4])  # gh_z
                ghn_c = wpool.tile([H, BW], F32, tag=f"ghn{s}", name=f"ghn{s}")
                nc.vector.tensor_copy(ghn_c[:], rGHN)
                r_t = wpool.tile([H, BW], F32, tag=f"r{s}", name=f"r{s}")
                nc.scalar.activation(r_t[:], rR, AF.Sigmoid, bias=rbias[:])
                yield
                np1 = wpool.tile([H, BW], F32, tag=f"np1{s}", name=f"np1{s}")
                if use_bhhn:
                    nc.vector.scalar_tensor_tensor(
                        np1[:], ghn_c[:], bhhn[:], r_t[:], ALU.add, ALU.mult
                    )
                else:
                    nc.vector.tensor_mul(np1[:], r_t[:], ghn_c[:])
                z_t = wpool.tile([H, BW], F32, tag=f"z{s}", name=f"z{s}")
                nc.scalar.activation(z_t[:], rZ, AF.Sigmoid, bias=zbias[:])
                npre = wpool.tile([H, BW], F32, tag=f"npre{s}", name=f"npre{s}")
                nc.vector.tensor_add(npre[:], np1[:], rGIN)
                n_t = wpool.tile([H, BW], F32, tag=f"n{s}", name=f"n{s}")
                nc.scalar.activation(n_t[:], npre[:], AF.Tanh, bias=nbias[:])
                yield
                zm1 = wpool.tile([H, BW], F32, tag=f"zm1{s}", name=f"zm1{s}")
                nc.vector.tensor_scalar(zm1[:], z_t[:], -1.0, 1.0, ALU.mult, ALU.add)
                zh = wpool.tile([H, BW], F32, tag=f"zh{s}", name=f"zh{s}")
                nc.vector.tensor_mul(zh[:], z_t[:], h[s][:])
                t3 = wpool.tile([H, BW], F32, tag=f"t3{s}", name=f"t3{s}")
                nc.vector.tensor_mul(t3[:], zm1[:], n_t[:])
                hn = hpool.tile([H, BW], MMDT, tag=f"h{s}", name=f"h{s}")
                nc.vector.tensor_add(hn[:], t3[:], zh[:])
                h[s] = hn
                mm(pa[0:NC_OUT, BW : 2 * BW], fcT[:], h[s][:], True, True)
                ot = opool.tile([NC_OUT, BW], F32, tag=f"o{s}", name=f"o{s}")
                nc.vector.tensor_scalar_add(ot[:], pa[0:NC_OUT, BW : 2 * BW], fcb[:])
                nc.sync.dma_start(out=outT[t][:, o : o + BW], in_=ot[:])
                yield

            for t in range(ts):
                xt_next = None
                if t + 1 < ts:
                    xt_next = xpool.tile([D_IN, B_LOC], MMDT, tag="xt", name="xt")
                    nc.sync.dma_start(out=xt_next[:], in_=d["xT"][t + 1])
                gens = [stream_step(s, t, xt_cur) for s in range(NS)]
                live = list(gens)
                while live:
                    nxt = []
                    for gen in live:
                        try:
                            next(gen)
                            nxt.append(gen)
                        except StopIteration:
                            pass
                    live = nxt
                if xt_next is not None:
                    xt_cur = xt_next

    nc.compile()
    return nc


def _prep_inputs(x, t, ode_w1, ode_b1, ode_w2, ode_b2, w_ih, w_hh, b_ih, b_hh,
                 fc_w, fc_b, ts):
    f64 = np.float64
    dts = np.asarray(t, f64)[1:] - np.asarray(t, f64)[:-1]
    dt = float(np.mean(dts))
    sub = dt / N_SUB
    c2 = 0.5 * sub
    c4 = sub
    d1 = sub / 6.0
    d2 = sub / 3.0

    w1 = np.asarray(ode_w1, f64)   # [50, 128]
    b1 = np.asarray(ode_b1, f64)   # [50]
    w2 = np.asarray(ode_w2, f64)   # [128, 50]
    b2 = np.asarray(ode_b2, f64)   # [128]

    W12 = w1 @ w2                  # [50, 50]
    w1b2 = w1 @ b2                 # [50]

    def f32c(a):
        return np.ascontiguousarray(a, dtype=np.float32)

    com = {
        "w1T": f32c(w1.T),
        "w12c2": f32c(np.concatenate([c2 * W12.T, (c2 * w1b2)[None, :]], 0)),
        "w12c4": f32c(np.concatenate([c4 * W12.T, (c4 * w1b2)[None, :]], 0)),
        "w2d1": f32c(np.concatenate([d1 * w2.T, (d1 * b2)[None, :]], 0)),
        "w2d2": f32c(np.concatenate([d2 * w2.T, (d2 * b2)[None, :]], 0)),
        "whhT": f32c(np.asarray(w_hh).T),
        "wihT": f32c(np.asarray(w_ih).T),
        "fcT": f32c(np.asarray(fc_w).T),
        "b1v": f32c(b1.reshape(MLP_H, 1)),
        "rbias": f32c((np.asarray(b_ih, f64)[0:H] + np.asarray(b_hh, f64)[0:H]).reshape(H, 1)),
        "zbias": f32c((np.asarray(b_ih, f64)[H:2*H] + np.asarray(b_hh, f64)[H:2*H]).reshape(H, 1)),
        "nbias": f32c(np.asarray(b_ih)[2*H:3*H].reshape(H, 1)),
        "bhhn": f32c(np.asarray(b_hh)[2*H:3*H].reshape(H, 1)),
        "fcb": f32c(np.asarray(fc_b).reshape(NC_OUT, 1)),
    }
    com["ones32"] = np.ones((32, B_LOC // NS), np.float32)
    com["zerosH"] = np.zeros((H, B_LOC // NS), np.float32)
    xnp = np.asarray(x, np.float32)
    in_maps = []
    for i in range(N_CORES):
        xi = xnp[:ts, i * B_LOC : (i + 1) * B_LOC, :]        # [ts, 256, 64]
        m = dict(com)
        m["xT"] = np.ascontiguousarray(xi.transpose(0, 2, 1))  # [ts, 64, 256]
        in_maps.append(m)
    use_bhhn = bool(np.any(np.asarray(b_hh)[2*H:3*H]))
    return in_maps, use_bhhn


def _run(inputs, ts=TS_FULL, trace=False):
    global LAST_EXEC_NS
    in_maps, use_bhhn = _prep_inputs(ts=ts, **inputs)
    key = (ts, use_bhhn)
    if key not in _BUILT:
        _BUILT[key] = _build_nc(ts, use_bhhn)
    nc = _BUILT[key]
    try:
        res = run_bass_kernel_spmd(nc, in_maps, list(range(N_CORES)), trace=trace)
    except ModuleNotFoundError:
        res = run_bass_kernel_spmd(nc, in_maps, list(range(N_CORES)), trace=False)
    LAST_EXEC_NS = res.exec_time_ns
    out = np.empty((ts, B_FULL, NC_OUT), np.float32)
    for i in range(N_CORES):
        out[:, i * B_LOC : (i + 1) * B_LOC, :] = res.results[i]["outT"].transpose(0, 2, 1)
    return out


def kernel(**inputs):
    return _run(inputs, ts=TS_FULL)


# revision 14
# speedup vs baseline: 1.0803x; 1.0803x over previous
"""Bass/Trainium2 kernel for nn_BaseODERNN (ODE-RNN: RK4 ODE solve + GRUCell + fc per step).

Strategy:
  - Pure data parallel over batch B=2048 -> 8 cores x 256.
  - Per core, batch is optionally split into NS interleaved "streams" whose
    dependency chains fill each other's engine-latency gaps.
  - Everything is kept in [feature, batch] layout so H=128 sits on SBUF
    partitions; x is pre-transposed on the host, output is produced transposed
    and fixed up on the host.
  - RK4 stage algebra is folded:
      u_1 = w1 @ h + b1
      u_{i+1} = u_1 + c_i * (W12 @ a_i + w1 @ b2),   W12 = w1 @ w2, a_i = tanh(u_i)
      h   += sum_i d_i * (w2 @ a_i + b2)
    so each stage is one PSUM-accumulated matmul + one tanh (bias folded into
    the ACT bias vector / augmented ones-row of a_i).
  - GRU: gi (from x_t) and gh (from h) accumulate into shared PSUM banks per
    gate; sigmoid/tanh read PSUM directly with folded biases.
  - NOTE: matmul start=True clears the WHOLE psum bank -> exactly one
    start=True per bank "era".
  - Matmuls optionally run as float32r (bitcast views): at moving-dim 256 the
    PE streams 1 cycle/col vs 4 for fp32.
"""

import os

import numpy as np

import concourse.bass as bass
import concourse.bacc as bacc
import concourse.mybir as mybir
from concourse import tile
from concourse.bass_utils import run_bass_kernel_spmd

F32 = mybir.dt.float32
F32R = mybir.dt.float32r
AF = mybir.ActivationFunctionType
ALU = mybir.AluOpType

T_FULL, B_FULL, D_IN, H, NC_OUT = 200, 2048, 64, 128, 32
MLP_H = 50
N_SUB = 4
N_CORES = 8
B_LOC = B_FULL // N_CORES   # 256
TS_FULL = T_FULL - 1        # 199 scan steps

NS = int(os.environ.get("K_NS", "1"))       # streams per core
USE_F32R = os.environ.get("K_F32R", "1") == "1"
BW = B_LOC // NS

LAST_EXEC_NS = None

_BUILT = {}


def _build_nc(ts, use_bhhn):
    nc = bacc.Bacc(
        "TRN2",
        target_bir_lowering=False,
        debug=False,
        num_devices=N_CORES,
        enable_asserts=False,
    )

    d = {}

    MMDT_D = F32R if USE_F32R else F32

    def din(name, shape, dt_=F32):
        d[name] = nc.dram_tensor(name, list(shape), dt_, kind="ExternalInput").ap()

    din("xT", (ts, D_IN, B_LOC), MMDT_D)
    din("w1T", (H, MLP_H), MMDT_D)
    din("w12c2", (MLP_H + 1, MLP_H), MMDT_D)
    din("w12c4", (MLP_H + 1, MLP_H), MMDT_D)
    din("w12d1", (MLP_H + 1, MLP_H), MMDT_D)
    din("w12d2", (MLP_H + 1, MLP_H), MMDT_D)
    din("w2d1", (MLP_H + 1, H), MMDT_D)
    din("w2d2", (MLP_H + 1, H), MMDT_D)
    din("whhT", (H, 3 * H), MMDT_D)
    din("wihT", (D_IN, 3 * H), MMDT_D)
    din("fcT", (H, NC_OUT), MMDT_D)
    din("b1v", (MLP_H, 1))
    din("rbias", (H, 1))
    din("zbias", (H, 1))
    din("nbias", (H, 1))
    din("bhhn", (H, 1))
    din("fcb", (NC_OUT, 1))
    din("ones32", (32, BW), MMDT_D)
    din("zerosH", (H, BW), MMDT_D)
    outT = nc.dram_tensor("outT", [ts, NC_OUT, B_LOC], F32, kind="ExternalOutput").ap()

    MMDT = F32R if USE_F32R else F32

    def mm(out, lhsT, rhs, start, stop):
        nc.tensor.matmul(out, lhsT, rhs, start=start, stop=stop)

    with tile.TileContext(nc) as tc:
        with (
            tc.tile_pool(name="const", bufs=1) as cpool,
            tc.tile_pool(name="xtp", bufs=2) as xpool,
            tc.tile_pool(name="hp", bufs=2) as hpool,
            tc.tile_pool(name="work", bufs=2) as wpool,
            tc.tile_pool(name="outp", bufs=3) as opool,
            tc.tile_pool(name="ps", bufs=1, space=bass.MemorySpace.PSUM) as pspool,
        ):
            def const_tile(name, shape, dt_=F32):
                t_ = cpool.tile(list(shape), dt_, tag=name, name=name)
                nc.sync.dma_start(out=t_[:], in_=d[name][:])
                return t_

            w1T = const_tile("w1T", (H, MLP_H), MMDT)
            w12c2 = const_tile("w12c2", (MLP_H + 1, MLP_H), MMDT)
            w12c4 = const_tile("w12c4", (MLP_H + 1, MLP_H), MMDT)
            w12d1 = const_tile("w12d1", (MLP_H + 1, MLP_H), MMDT)
            w12d2 = const_tile("w12d2", (MLP_H + 1, MLP_H), MMDT)
            w2d1 = const_tile("w2d1", (MLP_H + 1, H), MMDT)
            w2d2 = const_tile("w2d2", (MLP_H + 1, H), MMDT)
            whhT = const_tile("whhT", (H, 3 * H), MMDT)
            wihT = const_tile("wihT", (D_IN, 3 * H), MMDT)
            fcT = const_tile("fcT", (H, NC_OUT), MMDT)
            b1v = const_tile("b1v", (MLP_H, 1))
            rbias = const_tile("rbias", (H, 1))
            zbias = const_tile("zbias", (H, 1))
            nbias = const_tile("nbias", (H, 1))
            bhhn = const_tile("bhhn", (H, 1))
            fcb = const_tile("fcb", (NC_OUT, 1))

            # per-stream persistent a-tiles with a constant ones-row (bias row)
            atiles = []
            for s in range(NS):
                row = []
                for i in range(4):
                    a_ = cpool.tile([64, BW], MMDT, tag=f"a{i}s{s}", name=f"a{i}s{s}")
                    # ones "bias row" at partition 50 via DMA (memset can't target
                    # f32r and needs 32-aligned bases): rows [32:64) get 1.0;
                    # tanh rewrites [0:50) and rows 51+ are never read.
                    nc.sync.dma_start(out=a_[32:64, :], in_=d["ones32"][:])
                    row.append(a_)
                atiles.append(row)

            V = [
                [pspool.tile([MLP_H, BW], F32, tag=f"V{j}s{s}", name=f"V{j}s{s}")
                 for j in range(2)]
                for s in range(NS)
            ]
            V1 = [
                [pspool.tile([MLP_H, BW], F32, tag=f"W{j}s{s}", name=f"W{j}s{s}")
                 for j in range(2)]
                for s in range(NS)
            ]
            psafc = [pspool.tile([H, 2 * BW], F32, tag=f"pa{s}", name=f"pa{s}")
                     for s in range(NS)]
            # GRU gate psum regions: 4 x [H, BW] per stream.
            # BW=128: all four fit in one bank (one start=True era per step).
            # BW=256: two banks (r|z and ghn|gin), each with its own era.
            # flags per region: (gi_start, gi_stop, gh_start, gh_stop)
            gregs = []
            for s in range(NS):
                if BW == 128:
                    # all four regions share one bank; gi_n's start=True is the
                    # single whole-bank-clearing era start
                    g = pspool.tile([H, 4 * BW], F32, tag=f"g{s}", name=f"g{s}")
                    gregs.append({
                        "r": (g[:, 0:BW], False, False, False, False),
                        "z": (g[:, BW:2*BW], False, False, False, True),
                        "ghn": (g[:, 2*BW:3*BW], False, False, False, False),
                        "gin": (g[:, 3*BW:4*BW], True, False, None, None),
                    })
                else:
                    # one bank per gate; gi_n shares grN with ghn (evacuated to
                    # SBUF before the ghn era restarts the bank)
                    grR = pspool.tile([H, BW], F32, tag=f"grR{s}", name=f"grR{s}")
                    grZ = pspool.tile([H, BW], F32, tag=f"grZ{s}", name=f"grZ{s}")
                    grN = pspool.tile([H, BW], F32, tag=f"grN{s}", name=f"grN{s}")
                    gregs.append({
                        "r": (grR[:], True, False, False, True),
                        "z": (grZ[:], True, False, False, True),
                        "ghn": (grN[:], True, True, None, None),
                        "gin": (grN[:], True, True, None, None),
                    })

            # hidden state, zero-initialized
            h = []
            for s in range(NS):
                h0 = hpool.tile([H, BW], MMDT, tag=f"h{s}", name=f"h{s}")
                nc.sync.dma_start(out=h0[:], in_=d["zerosH"][:])
                h.append(h0)

            xt_cur = xpool.tile([D_IN, B_LOC], MMDT, tag="xt", name="xt")
            nc.sync.dma_start(out=xt_cur[:], in_=d["xT"][0])

            def stream_step(s, t, xt):
                o = s * BW
                a = atiles[s]
                va, vb = V[s]
                pa = psafc[s]
                gr = gregs[s]
                rR, rZ, rGHN, rGIN = gr["r"][0], gr["z"][0], gr["ghn"][0], gr["gin"][0]

                # gi matmuls: the designated region starts its bank's era
                mm(rGIN, wihT[:, 2 * H : 3 * H], xt[:, o : o + BW],
                   gr["gin"][1], gr["gin"][2])
                gin_c = wpool.tile([H, BW], F32, tag=f"gin{s}", name=f"gin{s}")
                nc.vector.tensor_copy(gin_c[:], rGIN)
                mm(rR, wihT[:, 0:H], xt[:, o : o + BW], gr["r"][1], gr["r"][2])
                mm(rZ, wihT[:, H : 2 * H], xt[:, o : o + BW], gr["z"][1], gr["z"][2])
                yield

                w1s = V1[s]
                for _k in range(N_SUB):
                    v1c = w1s[_k % 2]
                    v1n = w1s[(_k + 1) % 2] if _k < N_SUB - 1 else None
                    if _k == 0:
                        # step start: V1 = w1 @ h' (single-mm era)
                        mm(v1c[:], w1T[:], h[s][:], True, True)
                    # stage 2-4 bases + next-substep V1 base (all off-chain)
                    mm(va[:], w1T[:], h[s][:], True, False)
                    if v1n is not None:
                        mm(v1n[:], w1T[:], h[s][:], True, False)
                    nc.scalar.activation(a[0][0:MLP_H, :], v1c[:], AF.Tanh, bias=b1v[:])
                    yield
                    mm(va[:], w12c2[:], a[0][0 : MLP_H + 1, :], False, True)
                    mm(pa[:, 0:BW], w2d1[:], a[0][0 : MLP_H + 1, :], True, False)
                    if v1n is not None:
                        mm(v1n[:], w12d1[:], a[0][0 : MLP_H + 1, :], False, False)
                    mm(vb[:], w1T[:], h[s][:], True, False)      # stage3 base
                    nc.scalar.activation(a[1][0:MLP_H, :], va[:], AF.Tanh, bias=b1v[:])
                    yield
                    mm(vb[:], w12c2[:], a[1][0 : MLP_H + 1, :], False, True)
                    mm(pa[:, 0:BW], w2d2[:], a[1][0 : MLP_H + 1, :], False, False)
                    if v1n is not None:
                        mm(v1n[:], w12d2[:], a[1][0 : MLP_H + 1, :], False, False)
                    mm(va[:], w1T[:], h[s][:], True, False)      # stage4 base
                    nc.scalar.activation(a[2][0:MLP_H, :], vb[:], AF.Tanh, bias=b1v[:])
                    yield
                    mm(va[:], w12c4[:], a[2][0 : MLP_H + 1, :], False, True)
                    mm(pa[:, 0:BW], w2d2[:], a[2][0 : MLP_H + 1, :], False, False)
                    if v1n is not None:
                        mm(v1n[:], w12d2[:], a[2][0 : MLP_H + 1, :], False, False)
                    nc.scalar.activation(a[3][0:MLP_H, :], va[:], AF.Tanh, bias=b1v[:])
                    yield
                    if v1n is not None:
                        # chain-critical: next substep's tanh1 waits only this
                        mm(v1n[:], w12d1[:], a[3][0 : MLP_H + 1, :], False, True)
                    mm(pa[:, 0:BW], w2d1[:], a[3][0 : MLP_H + 1, :], False, True)
                    hn = hpool.tile([H, BW], MMDT, tag=f"h{s}", name=f"h{s}")
                    nc.vector.tensor_add(hn[:], h[s][:], pa[:, 0:BW])
                    h[s] = hn
                    yield

                # GRU
                mm(rGHN, whhT[:, 2 * H : 3 * H], h[s][:],
                   gr["ghn"][1], gr["ghn"][2])                           # gh_n
                mm(rR, whhT[:, 0:H], h[s][:], gr["r"][3], gr["r"][4])    # gh_r
                mm(rZ, whhT[:, H : 2 * H], h[s][:], gr["z"][3], gr["z"][4])  # gh_z
                ghn_c = wpool.tile([H, BW], F32, tag=f"ghn{s}", name=f"ghn{s}")
                nc.vector.tensor_copy(ghn_c[:], rGHN)
                r_t = wpool.tile([H, BW], F32, tag=f"r{s}", name=f"r{s}")
                nc.scalar.activation(r_t[:], rR, AF.Sigmoid, bias=rbias[:])
                yield
                np1 = wpool.tile([H, BW], F32, tag=f"np1{s}", name=f"np1{s}")
                if use_bhhn:
                    nc.vector.scalar_tensor_tensor(
                        np1[:], ghn_c[:], bhhn[:], r_t[:], ALU.add, ALU.mult
                    )
                else:
                    nc.vector.tensor_mul(np1[:], r_t[:], ghn_c[:])
                z_t = wpool.tile([H, BW], F32, tag=f"z{s}", name=f"z{s}")
                nc.scalar.activation(z_t[:], rZ, AF.Sigmoid, bias=zbias[:])
                npre = wpool.tile([H, BW], F32, tag=f"npre{s}", name=f"npre{s}")
                nc.vector.tensor_add(npre[:], np1[:], gin_c[:])
                n_t = wpool.tile([H, BW], F32, tag=f"n{s}", name=f"n{s}")
                nc.scalar.activation(n_t[:], npre[:], AF.Tanh, bias=nbias[:])
                yield
                zm1 = wpool.tile([H, BW], F32, tag=f"zm1{s}", name=f"zm1{s}")
                nc.vector.tensor_scalar(zm1[:], z_t[:], -1.0, 1.0, ALU.mult, ALU.add)
                zh = wpool.tile([H, BW], F32, tag=f"zh{s}", name=f"zh{s}")
                nc.vector.tensor_mul(zh[:], z_t[:], h[s][:])
                t3 = wpool.tile([H, BW], F32, tag=f"t3{s}", name=f"t3{s}")
                nc.vector.tensor_mul(t3[:], zm1[:], n_t[:])
                hn = hpool.tile([H, BW], MMDT, tag=f"h{s}", name=f"h{s}")
                nc.vector.tensor_add(hn[:], t3[:], zh[:])
                h[s] = hn
                mm(pa[0:NC_OUT, BW : 2 * BW], fcT[:], h[s][:], True, True)
                ot = opool.tile([NC_OUT, BW], F32, tag=f"o{s}", name=f"o{s}")
                nc.vector.tensor_scalar_add(ot[:], pa[0:NC_OUT, BW : 2 * BW], fcb[:])
                nc.sync.dma_start(out=outT[t][:, o : o + BW], in_=ot[:])
                yield

            for t in range(ts):
                xt_next = None
                if t + 1 < ts:
                    xt_next = xpool.tile([D_IN, B_LOC], MMDT, tag="xt", name="xt")
                    nc.sync.dma_start(out=xt_next[:], in_=d["xT"][t + 1])
                gens = [stream_step(s, t, xt_cur) for s in range(NS)]
                live = list(gens)
                while live:
                    nxt = []
                    for gen in live:
                        try:
                            next(gen)
                            nxt.append(gen)
                        except StopIteration:
                            pass
                    live = nxt
                if xt_next is not None:
                    xt_cur = xt_next

    nc.compile()
    return nc


def _prep_inputs(x, t, ode_w1, ode_b1, ode_w2, ode_b2, w_ih, w_hh, b_ih, b_hh,
                 fc_w, fc_b, ts):
    f64 = np.float64
    dts = np.asarray(t, f64)[1:] - np.asarray(t, f64)[:-1]
    dt = float(np.mean(dts))
    sub = dt / N_SUB
    c2 = 0.5 * sub
    c4 = sub
    d1 = sub / 6.0
    d2 = sub / 3.0

    w1 = np.asarray(ode_w1, f64)   # [50, 128]
    b1 = np.asarray(ode_b1, f64)   # [50]
    w2 = np.asarray(ode_w2, f64)   # [128, 50]
    b2 = np.asarray(ode_b2, f64)   # [128]

    W12 = w1 @ w2                  # [50, 50]
    w1b2 = w1 @ b2                 # [50]

    def f32c(a):
        return np.ascontiguousarray(a, dtype=np.float32)

    com = {
        "w1T": f32c(w1.T),
        "w12c2": f32c(np.concatenate([c2 * W12.T, (c2 * w1b2)[None, :]], 0)),
        "w12c4": f32c(np.concatenate([c4 * W12.T, (c4 * w1b2)[None, :]], 0)),
        "w12d1": f32c(np.concatenate([d1 * W12.T, (d1 * w1b2)[None, :]], 0)),
        "w12d2": f32c(np.concatenate([d2 * W12.T, (d2 * w1b2)[None, :]], 0)),
        "w2d1": f32c(np.concatenate([d1 * w2.T, (d1 * b2)[None, :]], 0)),
        "w2d2": f32c(np.concatenate([d2 * w2.T, (d2 * b2)[None, :]], 0)),
        "whhT": f32c(np.asarray(w_hh).T),
        "wihT": f32c(np.asarray(w_ih).T),
        "fcT": f32c(np.asarray(fc_w).T),
        "b1v": f32c(b1.reshape(MLP_H, 1)),
        "rbias": f32c((np.asarray(b_ih, f64)[0:H] + np.asarray(b_hh, f64)[0:H]).reshape(H, 1)),
        "zbias": f32c((np.asarray(b_ih, f64)[H:2*H] + np.asarray(b_hh, f64)[H:2*H]).reshape(H, 1)),
        "nbias": f32c(np.asarray(b_ih)[2*H:3*H].reshape(H, 1)),
        "bhhn": f32c(np.asarray(b_hh)[2*H:3*H].reshape(H, 1)),
        "fcb": f32c(np.asarray(fc_b).reshape(NC_OUT, 1)),
    }
    com["ones32"] = np.ones((32, B_LOC // NS), np.float32)
    com["zerosH"] = np.zeros((H, B_LOC // NS), np.float32)
    xnp = np.asarray(x, np.float32)
    in_maps = []
    for i in range(N_CORES):
        xi = xnp[:ts, i * B_LOC : (i + 1) * B_LOC, :]        # [ts, 256, 64]
        m = dict(com)
        m["xT"] = np.ascontiguousarray(xi.transpose(0, 2, 1))  # [ts, 64, 256]
        in_maps.append(m)
    use_bhhn = bool(np.any(np.asarray(b_hh)[2*H:3*H]))
    return in_maps, use_bhhn


def _run(inputs, ts=TS_FULL, trace=False):
    global LAST_EXEC_NS
    in_maps, use_bhhn = _prep_inputs(ts=ts, **inputs)
    key = (ts, use_bhhn)
    if key not in _BUILT:
        _BUILT[key] = _build_nc(ts, use_bhhn)
    nc = _BUILT[key]
    try:
        res = run_bass_kernel_spmd(nc, in_maps, list(range(N_CORES)), trace=trace)
    except ModuleNotFoundError:
        res = run_bass_kernel_spmd(nc, in_maps, list(range(N_CORES)), trace=False)
    LAST_EXEC_NS = res.exec_time_ns
    out = np.empty((ts, B_FULL, NC_OUT), np.float32)
    for i in range(N_CORES):
        out[:, i * B_LOC : (i + 1) * B_LOC, :] = res.results[i]["outT"].transpose(0, 2, 1)
    return out


def kernel(**inputs):
    return _run(inputs, ts=TS_FULL)
